# revision 21
# baseline (speedup 1.0000x reference)
"""Multi-head causal self-attention (B=2, S=2048, D=1024, H=16) on 8 TRN2 NeuronCores.

Sharding: data-parallel over batch (2) x tensor-parallel over heads (4 groups of
4 heads). Each core computes Q/K/V projections for its 4 heads, causal
flash-style attention (scores kept transposed [k, q] so no on-chip transposes
are needed), and a partial output projection against its row-slice of W_O.
Host sums the 4 partials per batch and adds the output bias.

v2: bf16 matmul operands (less PE power -> less duty-cycle throttle, half the
DMA/LDWEIGHTS bytes), per-k-block software-pipelined attention steps (the PE
queue is [QK(s), PV(s-1)] so PV never waits on the softmax exp), PSUM
rebalanced (4x one-bank score tiles + 2x double-head pv accumulators), and the
softmax normalize chain (psum evac -> den bounce -> [128,8] reciprocal ->
stride-0 broadcast -> multiply) deferred several steps off the critical path.
"""

import contextlib
import sys

import numpy as np

sys.path.insert(0, "/opt/trn_rl_repo")

import concourse.bass as bass  # noqa: E402
import concourse.tile as tile  # noqa: E402
from concourse import bacc, mybir  # noqa: E402
from concourse.bass_utils import run_bass_kernel_spmd  # noqa: E402

F32 = mybir.dt.float32
BF16 = mybir.dt.bfloat16
AF = mybir.ActivationFunctionType
ALU = mybir.AluOpType

B, S, D, H = 2, 2048, 1024, 16
DH = D // H          # 64
TPG = 4              # tensor-parallel groups
HPC = H // TPG       # 4 heads per core
CH = HPC * DH        # 256 channels per core
CHA = CH + HPC       # 260: V channels augmented with a ones column per head
NEG = -1.0e9
N_CORES = 8

_PROG = None  # cached compiled Bass program


def _build_program():
    nc = bacc.Bacc("TRN2", target_bir_lowering=False, debug=False,
                   num_devices=N_CORES)

    # weights arrive chunk-interleaved ([partition, chunk, col]) so each
    # DMA packet is a full 4KB per-partition row
    xT = nc.dram_tensor("xT", [D, S], BF16, kind="ExternalInput").ap()
    wq = nc.dram_tensor("wq", [128, 8 * CH], BF16, kind="ExternalInput").ap()
    wk = nc.dram_tensor("wk", [128, 8 * CH], BF16, kind="ExternalInput").ap()
    wv = nc.dram_tensor("wv", [128, 8 * CHA], BF16, kind="ExternalInput").ap()
    wo = nc.dram_tensor("wo", [128, 2 * D], BF16, kind="ExternalInput").ap()
    bq = nc.dram_tensor("bq", [128, 2], F32, kind="ExternalInput").ap()
    bk = nc.dram_tensor("bk", [128, 2], F32, kind="ExternalInput").ap()
    tri = nc.dram_tensor("tri", [128, 1024], F32, kind="ExternalInput").ap()
    onesf = nc.dram_tensor("onesf", [1, 64], F32, kind="ExternalInput").ap()
    out = nc.dram_tensor("out", [S, D], F32, kind="ExternalOutput").ap()

    NQ = S // 512    # 4 q-blocks of 512
    NT = S // 128    # 16 s-tiles / k-blocks

    with tile.TileContext(nc) as tc, contextlib.ExitStack() as ctx:
        const = ctx.enter_context(tc.tile_pool(name="const", bufs=1))
        qt = const.tile([128, 2, S], BF16)     # Q^T/8 (+bq/8): chunk m = heads 2m,2m+1
        kt = const.tile([128, 2, S], BF16)     # K^T (+bk)
        va = const.tile([128, NT, CHA], BF16)  # V augmented: [s, head-major 65-col blocks]
        otn = const.tile([128, 2, S], BF16)    # normalized attention out, transposed
        tri_t = const.tile([128, 1024], F32)
        ones64f = const.tile([1, 64], F32)
        bq_t = const.tile([128, 2], F32)
        bk_t = const.tile([128, 2], F32)
        wo_t = const.tile([128, 2, D], BF16)

        # ---- phase 1: projections --------------------------------------
        with tc.tile_pool(name="proj", bufs=1) as proj, \
             tc.tile_pool(name="pqk", bufs=8, space="PSUM") as pqk:
            xt = proj.tile([128, 8, S], BF16)
            wq_t = proj.tile([128, 8, CH], BF16)
            wk_t = proj.tile([128, 8, CH], BF16)
            wv_t = proj.tile([128, 8, CHA], BF16)
            xTr = xT.rearrange("(a p) s -> a p s", p=128)
            # x streams chunk-by-chunk (4KB per-partition packets); the
            # first matmul needs only wq + the first quarter of x chunk 0
            nc.sync.dma_start(wq_t, wq)
            for n in range(NQ):
                nc.sync.dma_start(xt[:, 0, n * 512:(n + 1) * 512],
                                  xTr[0][:, n * 512:(n + 1) * 512])
            nc.sync.dma_start(bq_t, bq)
            nc.sync.dma_start(wk_t, wk)
            nc.sync.dma_start(bk_t, bk)
            for c in range(1, 8):
                nc.sync.dma_start(xt[:, c, :], xTr[c])
            nc.sync.dma_start(wv_t, wv)
            # non-projection constants after the projection-critical stream
            nc.sync.dma_start(ones64f, onesf)
            nc.sync.dma_start(tri_t, tri)
            nc.sync.dma_start(wo_t, wo)

            # preload the ACT exp table set while ACT is otherwise idle
            nc.scalar.activation(ones64f, ones64f, AF.Exp)

            def qk_pass(m):
                # c-outer: 8 open psum groups (Q n0..3, K n0..3) accumulate
                # as each x chunk arrives; first matmul needs only chunk 0
                groups = []
                for w_t, dst, bias_t, scaled in ((wq_t, qt, bq_t, True),
                                                 (wk_t, kt, bk_t, False)):
                    for n in range(NQ):
                        ps = pqk.tile([128, 512], F32, tag="pqk", name="pqk")
                        groups.append((ps, w_t, dst, bias_t, scaled, n))
                for c in range(8):
                    for ps, w_t, dst, bias_t, scaled, n in groups:
                        nc.tensor.matmul(
                            ps, (w_t[:, c, m * 128:(m + 1) * 128]),
                            (xt[:, c, n * 512:(n + 1) * 512]),
                            start=(c == 0), stop=(c == 7))
                        if c == 7:
                            # evacuate on ACT; fold 1/sqrt(DH) into Q
                            nc.scalar.activation(
                                dst[:, m, n * 512:(n + 1) * 512], ps,
                                AF.Identity, bias=bias_t[:, m:m + 1],
                                scale=0.125 if scaled else 1.0)

            def v_proj():
                # V (not transposed): stationary = x^T tile, moving = wv_aug
                # (the V bias folds into the host-side output bias because
                # softmax weights sum to 1: o_norm += bv  =>  out += bv @ Wo)
                for t in range(NT):
                    psv = pqk.tile([128, CHA], F32, tag="pqk", name="psv")
                    for c in range(8):
                        nc.tensor.matmul(
                            psv, (xt[:, c, t * 128:(t + 1) * 128]),
                            (wv_t[:, c, :]), start=(c == 0), stop=(c == 7))
                    nc.vector.tensor_copy(va[:, t, :], psv)
                    # the per-head denominator "ones" columns (65th of each
                    # head block; zero in wv_aug so psv has zeros there)
                    ones_view = bass.AP(
                        tensor=va.tensor, offset=va.offset + t * CHA + DH,
                        ap=[list(va.ap)[0], [DH + 1, HPC]])
                    nc.vector.memset(ones_view, 1.0)

            qk_pass(0)
            v_proj()
            qk_pass(1)

        # ---- phase 2: attention, software-pipelined per k-block --------
        # step = (j, p, kb): one 128-row k-block of scores for a 512-col
        # q-block, both packed head-halves (hh).  The PE queue is
        # [QK(s), PV(s-1)] so PV never waits on exp latency.  The softmax
        # normalize chain for each (j, p) unit is deferred several steps.
        steps = []
        for j in range(NQ):
            for p in range(2):
                nkb = 4 * (j + 1)
                for kb in range(nkb):
                    steps.append((j, p, kb, kb == 0, kb == nkb - 1))
        n_steps = len(steps)

        deferred = []   # (due_step, fn), non-decreasing due order

        def run_due(s):
            while deferred and deferred[0][0] <= s:
                deferred.pop(0)[1]()

        with tc.tile_pool(name="ovp", bufs=2) as ovp, \
             tc.tile_pool(name="rcp", bufs=2) as rcp, \
             tc.tile_pool(name="bcp", bufs=2) as bcp, \
             tc.tile_pool(name="dsp", bufs=4, space="DRAM") as dsp:

            def make_normalize(j, p, pv, s_end):
                # chain: evac pv->SBUF (frees PSUM) -> bounce den ->
                # [128,8] reciprocal -> bounce back -> stride-0 broadcast
                # -> per-head multiply into otn.
                qsl = slice(j * 512, (j + 1) * 512)
                state = {}

                def evac():
                    ov = ovp.tile([65, 1024], F32, tag="ov", name="ov")
                    nc.vector.tensor_copy(ov, pv)
                    drow = dsp.tile([1, 1024], F32, tag="ds", name="ds")
                    nc.sync.dma_start(drow, ov[64:65, :])
                    rin = rcp.tile([128, 2, 4], F32, tag="ri", name="ri")
                    din_src = bass.AP(
                        tensor=drow.tensor, offset=drow.offset,
                        ap=[[4, 128], [512, 2], [1, 4]])
                    nc.sync.dma_start(rin, din_src)
                    state["ov"] = ov
                    state["rin"] = rin

                def recip():
                    rout = rcp.tile([128, 2, 4], F32, tag="ro", name="ro")
                    nc.vector.reciprocal(rout, state["rin"])
                    rrow = dsp.tile([1, 1024], F32, tag="rr", name="rr")
                    rr_dst = bass.AP(
                        tensor=rrow.tensor, offset=rrow.offset,
                        ap=[[4, 128], [512, 2], [1, 4]])
                    nc.sync.dma_start(rr_dst, rout)
                    bcs = bcp.tile([64, 2, 512], F32, tag="bc", name="bc")
                    bc_src = bass.AP(
                        tensor=rrow.tensor, offset=rrow.offset,
                        ap=[[0, 64], [512, 2], [1, 512]])
                    nc.sync.dma_start(bcs, bc_src)
                    state["bcs"] = bcs

                def muls():
                    ov, bcs = state["ov"], state["bcs"]
                    for hh in range(2):
                        oh = hh * 64
                        nc.vector.tensor_mul(
                            otn[oh:oh + 64, p, qsl],
                            ov[0:64, hh * 512:(hh + 1) * 512],
                            bcs[:, hh, :])

                deferred.append((s_end + 2, evac))
                deferred.append((s_end + 3, recip))
                deferred.append((s_end + 5, muls))

            with tc.tile_pool(name="sm", bufs=4) as sm, \
                 tc.tile_pool(name="stp", bufs=4, space="PSUM") as stp, \
                 tc.tile_pool(name="pvp", bufs=2, space="PSUM") as pvp:
                # pair-batched emission: 4 QK matmuls (64-row-tiled mode,
                # h0/h64 tiles run concurrently) then 4 PV matmuls (full
                # 128x128 mode) — mode switches per pair instead of per
                # matmul, avoiding a PE drain on every instruction.
                assert len(steps) % 2 == 0
                prev_pair = []  # list of (pt, kb, first, last, pv, rel, w)
                for k2 in range(0, n_steps, 2):
                    run_due(k2)
                    pair = steps[k2:k2 + 2]
                    emitted = []
                    for (j, p, kb, first, last) in pair:
                        if first:
                            pv = pvp.tile([65, 1024], F32, tag="pv",
                                          name="pv")
                        rel = max(kb * 128 - j * 512, 0)
                        w = 512 - rel
                        qsl = slice(j * 512 + rel, (j + 1) * 512)
                        st = [stp.tile([128, 512], F32, tag="st",
                                       name=f"st{_hh}") for _hh in range(2)]
                        for hh in range(2):
                            oh = hh * 64
                            nc.tensor.matmul(
                                st[hh][:, 0:w],
                                (kt[oh:oh + 64, p, kb * 128:(kb + 1) * 128]),
                                (qt[oh:oh + 64, p, qsl]),
                                start=True, stop=True)
                        emitted.append((j, p, kb, first, last, pv, rel, w,
                                        st))
                    # previous pair's PVs (full-array mode, one switch)
                    for pr in prev_pair:
                        _emit_pv(nc, va, pr)
                    prev_pair = []
                    for (j, p, kb, first, last, pv, rel, w, st) in emitted:
                        diag = kb * 128 - j * 512 >= 0
                        pt = [None, None]
                        for hh in range(2):
                            if diag:
                                # causal staircase bias, first 128 cols
                                sl = st[hh][:, 0:128]
                                nc.vector.tensor_add(sl, sl,
                                                     tri_t[:, 512:640])
                            pt[hh] = sm.tile([128, 512], BF16, tag="pt",
                                             name=f"pt{hh}")
                            nc.scalar.activation(pt[hh][:, 0:w],
                                                 st[hh][:, 0:w], AF.Exp)
                        prev_pair.append((pt, p, kb, first, last, pv, rel,
                                          w))
                        if last:
                            make_normalize(j, p, pv, k2 + 1)
                for pr in prev_pair:
                    _emit_pv(nc, va, pr)
                # flush everything due so far (incl. the last unit's psum
                # evac) while the score/pv pools are still open
                run_due(n_steps + 1)

            # ---- phase 3: output projection (partial; host reduces) ----
            with tc.tile_pool(name="ost2", bufs=4) as ost2, \
                 tc.tile_pool(name="ops", bufs=4, space="PSUM") as ops:
                for t in range(NT):
                    if t == 4:
                        run_due(n_steps + 3)
                    if t == 8:
                        run_due(n_steps + 5)
                    # full 4KB output rows per DMA packet: evac both
                    # 512-wide psum halves into one [128, 1024] tile
                    so = ost2.tile([128, 1024], F32, tag="so", name="so")
                    for n in range(2):
                        ps = ops.tile([128, 512], F32, tag="ops", name="ops")
                        for c2 in range(2):
                            nc.tensor.matmul(
                                ps, (otn[:, c2, t * 128:(t + 1) * 128]),
                                (wo_t[:, c2, n * 512:(n + 1) * 512]),
                                start=(c2 == 0), stop=(c2 == 1))
                        nc.vector.tensor_copy(so[:, n * 512:(n + 1) * 512],
                                              ps)
                    nc.sync.dma_start(out[t * 128:(t + 1) * 128, :], so)

    nc.compile()
    return nc


def _emit_pv(nc, va, prev):
    pt, p, kb, first, last, pv, rel, w = prev
    for hh in range(2):
        h = 2 * p + hh
        nc.tensor.matmul(
            pv[:, hh * 512 + rel:(hh + 1) * 512],
            (va[:, kb, h * 65:h * 65 + 65]),
            (pt[hh][:, 0:w]),
            start=first, stop=last, skip_group_check=True)


def _tri_np():
    # staircase causal bias: tri[kk, x] = NEG if x < 512+kk else 0
    xs = np.arange(1024)[None, :]
    ks = np.arange(128)[:, None]
    return np.where(xs < 512 + ks, np.float32(NEG),
                    np.float32(0.0)).astype(np.float32)


def build_in_maps(x, Wq, bq, Wk, bk, Wv, bv, Wo):
    import ml_dtypes
    bf16 = ml_dtypes.bfloat16
    tri_np = _tri_np()
    xT_b = [np.ascontiguousarray(x[b].T).astype(bf16) for b in range(B)]
    in_maps = []
    for c in range(N_CORES):
        b, tp = divmod(c, TPG)
        sl = slice(tp * CH, (tp + 1) * CH)
        # V bias is folded into the host-side output bias (bv @ Wo); the
        # ones columns are memset on-device.
        wv_aug = np.zeros((D, CHA), dtype=np.float32)
        for h in range(HPC):
            hsl = slice(tp * CH + h * DH, tp * CH + (h + 1) * DH)
            wv_aug[:, h * 65:h * 65 + DH] = Wv[:, hsl]

        def chunked(w, nch):
            # [nch*128, cols] -> [128, nch*cols]: per-partition rows hold
            # all chunks contiguously so DMA packets are full 4KB rows
            cols = w.shape[1]
            return np.ascontiguousarray(
                w.reshape(nch, 128, cols).transpose(1, 0, 2)
            ).reshape(128, nch * cols).astype(bf16)

        in_maps.append({
            "xT": xT_b[b],
            "wq": chunked(np.ascontiguousarray(Wq[:, sl]), 8),
            "wk": chunked(np.ascontiguousarray(Wk[:, sl]), 8),
            "wv": chunked(wv_aug, 8),
            "wo": chunked(np.ascontiguousarray(Wo[sl, :]), 2),
            "bq": (bq[sl].astype(np.float32) * 0.125).reshape(2, 128).T.copy(),
            "bk": bk[sl].astype(np.float32).reshape(2, 128).T.copy(),
            "tri": tri_np,
            "onesf": np.ones((1, 64), dtype=np.float32),
        })
    return in_maps


def _get_program():
    global _PROG
    if _PROG is None:
        _PROG = _build_program()
    return _PROG


def kernel(x, mask, Wq, bq, Wk, bk, Wv, bv, Wo, bo):
    x = np.asarray(x, dtype=np.float32)
    mask = np.asarray(mask)
    Wq, Wk, Wv, Wo = (np.asarray(w, dtype=np.float32)
                      for w in (Wq, Wk, Wv, Wo))
    bq, bk, bv, bo = (np.asarray(b, dtype=np.float32)
                      for b in (bq, bk, bv, bo))
    causal = bool(
        np.array_equal(mask != 0,
                       np.tril(np.ones((S, S), dtype=bool))))
    if not causal:
        # Fallback for non-causal masks: exact host computation.
        q = (x @ Wq + bq).reshape(B, S, H, DH).transpose(0, 2, 1, 3)
        k = (x @ Wk + bk).reshape(B, S, H, DH).transpose(0, 2, 1, 3)
        v = (x @ Wv + bv).reshape(B, S, H, DH).transpose(0, 2, 1, 3)
        attn = np.einsum("bhqd,bhkd->bhqk", q, k) / np.sqrt(np.float32(DH))
        attn = np.where(mask == 0, np.float32(-1e9), attn)
        attn = attn - attn.max(axis=-1, keepdims=True)
        e = np.exp(attn)
        p = e / e.sum(axis=-1, keepdims=True)
        o = np.einsum("bhqk,bhkd->bhqd", p, v)
        o = o.transpose(0, 2, 1, 3).reshape(B, S, D)
        return (o @ Wo + bo).astype(np.float32)

    nc = _get_program()
    in_maps = build_in_maps(x, Wq, bq, Wk, bk, Wv, bv, Wo)
    res = run_bass_kernel_spmd(nc, in_maps, core_ids=list(range(N_CORES)))
    out = np.zeros((B, S, D), dtype=np.float32)
    for c in range(N_CORES):
        out[c // TPG] += res.results[c]["out"]
    # softmax weights sum to 1, so the V bias contributes exactly bv @ Wo
    out += (bv.astype(np.float32) @ Wo) + bo.astype(np.float32)
    return out


# revision 25
# speedup vs baseline: 1.3289x; 1.3289x over previous
"""Multi-head causal self-attention (B=2, S=2048, D=1024, H=16) on 8 TRN2 NeuronCores.

Sharding: data-parallel over batch (2) x tensor-parallel over heads (4 groups of
4 heads). Each core computes Q/K/V projections for its 4 heads, causal
flash-style attention (scores kept transposed [k, q] so no on-chip transposes
are needed), and a partial output projection against its row-slice of W_O.
Host sums the 4 partials per batch and adds the output bias.

v2: bf16 matmul operands (less PE power -> less duty-cycle throttle, half the
DMA/LDWEIGHTS bytes), per-k-block software-pipelined attention steps (the PE
queue is [QK(s), PV(s-1)] so PV never waits on the softmax exp), PSUM
rebalanced (4x one-bank score tiles + 2x double-head pv accumulators), and the
softmax normalize chain (psum evac -> den bounce -> [128,8] reciprocal ->
stride-0 broadcast -> multiply) deferred several steps off the critical path.
"""

import contextlib
import sys

import numpy as np

sys.path.insert(0, "/opt/trn_rl_repo")

import concourse.bass as bass  # noqa: E402
import concourse.tile as tile  # noqa: E402
from concourse import bacc, mybir  # noqa: E402
from concourse.bass_utils import run_bass_kernel_spmd  # noqa: E402

F32 = mybir.dt.float32
BF16 = mybir.dt.bfloat16
AF = mybir.ActivationFunctionType
ALU = mybir.AluOpType

B, S, D, H = 2, 2048, 1024, 16
DH = D // H          # 64
TPG = 4              # tensor-parallel groups
HPC = H // TPG       # 4 heads per core
CH = HPC * DH        # 256 channels per core
CHA = CH + HPC       # 260: V channels augmented with a ones column per head
NEG = -1.0e9
N_CORES = 8

_PROG = None  # cached compiled Bass program


def _build_program():
    nc = bacc.Bacc("TRN2", target_bir_lowering=False, debug=False,
                   num_devices=N_CORES)

    # weights arrive chunk-interleaved ([partition, chunk, col]) so each
    # DMA packet is a full 4KB per-partition row
    xT = nc.dram_tensor("xT", [D, S], BF16, kind="ExternalInput").ap()
    wq = nc.dram_tensor("wq", [128, 8 * CH], BF16, kind="ExternalInput").ap()
    wk = nc.dram_tensor("wk", [128, 8 * CH], BF16, kind="ExternalInput").ap()
    wv = nc.dram_tensor("wv", [128, 8 * CHA], BF16, kind="ExternalInput").ap()
    wo = nc.dram_tensor("wo", [128, 2 * D], BF16, kind="ExternalInput").ap()
    bq = nc.dram_tensor("bq", [128, 2], F32, kind="ExternalInput").ap()
    bk = nc.dram_tensor("bk", [128, 2], F32, kind="ExternalInput").ap()
    tri = nc.dram_tensor("tri", [128, 1024], F32, kind="ExternalInput").ap()
    onesf = nc.dram_tensor("onesf", [1, 64], F32, kind="ExternalInput").ap()
    out = nc.dram_tensor("out", [S, D], F32, kind="ExternalOutput").ap()

    NQ = S // 512    # 4 q-blocks of 512
    NT = S // 128    # 16 s-tiles / k-blocks

    with tile.TileContext(nc) as tc, contextlib.ExitStack() as ctx:
        const = ctx.enter_context(tc.tile_pool(name="const", bufs=1))
        qt = const.tile([128, 2, S], BF16)     # Q^T/8 (+bq/8): chunk m = heads 2m,2m+1
        kt = const.tile([128, 2, S], BF16)     # K^T (+bk)
        va = const.tile([128, NT, CHA], BF16)  # V augmented: [s, head-major 65-col blocks]
        otn = const.tile([128, 2, S], BF16)    # normalized attention out, transposed
        tri_t = const.tile([128, 1024], F32)
        ones64f = const.tile([1, 64], F32)
        bq_t = const.tile([128, 2], F32)
        bk_t = const.tile([128, 2], F32)
        wo_t = const.tile([128, 2, D], BF16)

        # ---- phase 1: projections --------------------------------------
        with tc.tile_pool(name="proj", bufs=1) as proj, \
             tc.tile_pool(name="pqk", bufs=8, space="PSUM") as pqk:
            xt = proj.tile([128, 8, S], BF16)
            wq_t = proj.tile([128, 8, CH], BF16)
            wk_t = proj.tile([128, 8, CH], BF16)
            wv_t = proj.tile([128, 8, CHA], BF16)
            xTr = xT.rearrange("(a p) s -> a p s", p=128)
            # x streams chunk-by-chunk (4KB per-partition packets); the
            # first matmul needs only wq + the first quarter of x chunk 0
            nc.sync.dma_start(wq_t, wq)
            for n in range(NQ):
                nc.sync.dma_start(xt[:, 0, n * 512:(n + 1) * 512],
                                  xTr[0][:, n * 512:(n + 1) * 512])
            nc.sync.dma_start(bq_t, bq)
            nc.sync.dma_start(wk_t, wk)
            nc.sync.dma_start(bk_t, bk)
            for c in range(1, 8):
                nc.sync.dma_start(xt[:, c, :], xTr[c])
            nc.sync.dma_start(wv_t, wv)
            # non-projection constants after the projection-critical stream
            nc.sync.dma_start(ones64f, onesf)
            nc.sync.dma_start(tri_t, tri)
            nc.sync.dma_start(wo_t, wo)

            # preload the ACT exp table set while ACT is otherwise idle
            nc.scalar.activation(ones64f, ones64f, AF.Exp)

            def qk_pass(m):
                # c-outer: 8 open psum groups (Q n0..3, K n0..3) accumulate
                # as each x chunk arrives; first matmul needs only chunk 0
                groups = []
                for w_t, dst, bias_t, scaled in ((wq_t, qt, bq_t, True),
                                                 (wk_t, kt, bk_t, False)):
                    for n in range(NQ):
                        ps = pqk.tile([128, 512], F32, tag="pqk", name="pqk")
                        groups.append((ps, w_t, dst, bias_t, scaled, n))
                for c in range(8):
                    for ps, w_t, dst, bias_t, scaled, n in groups:
                        nc.tensor.matmul(
                            ps, (w_t[:, c, m * 128:(m + 1) * 128]),
                            (xt[:, c, n * 512:(n + 1) * 512]),
                            start=(c == 0), stop=(c == 7))
                        if c == 7:
                            # evacuate on ACT; fold 1/sqrt(DH) into Q
                            nc.scalar.activation(
                                dst[:, m, n * 512:(n + 1) * 512], ps,
                                AF.Identity, bias=bias_t[:, m:m + 1],
                                scale=0.125 if scaled else 1.0)

            def v_proj():
                # V (not transposed): stationary = x^T tile, moving = wv_aug
                # (the V bias folds into the host-side output bias because
                # softmax weights sum to 1: o_norm += bv  =>  out += bv @ Wo)
                for t in range(NT):
                    psv = pqk.tile([128, CHA], F32, tag="pqk", name="psv")
                    for c in range(8):
                        nc.tensor.matmul(
                            psv, (xt[:, c, t * 128:(t + 1) * 128]),
                            (wv_t[:, c, :]), start=(c == 0), stop=(c == 7))
                    nc.vector.tensor_copy(va[:, t, :], psv)
                    # the per-head denominator "ones" columns (65th of each
                    # head block; zero in wv_aug so psv has zeros there)
                    ones_view = bass.AP(
                        tensor=va.tensor, offset=va.offset + t * CHA + DH,
                        ap=[list(va.ap)[0], [DH + 1, HPC]])
                    nc.vector.memset(ones_view, 1.0)

            qk_pass(0)
            v_proj()
            qk_pass(1)

        # ---- phase 2: attention, software-pipelined per k-block --------
        # step = (j, p, kb): one 128-row k-block of scores for a 512-col
        # q-block, both packed head-halves (hh).  The PE queue is
        # [QK(s), PV(s-1)] so PV never waits on exp latency.  The softmax
        # normalize chain for each (j, p) unit is deferred several steps.
        steps = []
        for j in range(NQ):
            for p in range(2):
                nkb = 4 * (j + 1)
                for kb in range(nkb):
                    steps.append((j, p, kb, kb == 0, kb == nkb - 1))
        n_steps = len(steps)

        deferred = []   # (due_step, fn), non-decreasing due order

        def run_due(s):
            while deferred and deferred[0][0] <= s:
                deferred.pop(0)[1]()

        with tc.tile_pool(name="ovp", bufs=2) as ovp, \
             tc.tile_pool(name="rcp", bufs=2) as rcp, \
             tc.tile_pool(name="bcp", bufs=2) as bcp, \
             tc.tile_pool(name="dsp", bufs=4, space="DRAM") as dsp:

            def make_normalize(j, p, pv, s_end):
                # chain: evac pv->SBUF (frees PSUM) -> bounce den ->
                # [128,8] reciprocal -> bounce back -> stride-0 broadcast
                # -> per-head multiply into otn.
                qsl = slice(j * 512, (j + 1) * 512)
                state = {}

                def evac():
                    ov = ovp.tile([65, 1024], F32, tag="ov", name="ov")
                    nc.vector.tensor_copy(ov, pv)
                    drow = dsp.tile([1, 1024], F32, tag="ds", name="ds")
                    nc.sync.dma_start(drow, ov[64:65, :])
                    rin = rcp.tile([128, 2, 4], F32, tag="ri", name="ri")
                    din_src = bass.AP(
                        tensor=drow.tensor, offset=drow.offset,
                        ap=[[4, 128], [512, 2], [1, 4]])
                    nc.sync.dma_start(rin, din_src)
                    state["ov"] = ov
                    state["rin"] = rin

                def recip():
                    rout = rcp.tile([128, 2, 4], F32, tag="ro", name="ro")
                    nc.vector.reciprocal(rout, state["rin"])
                    rrow = dsp.tile([1, 1024], F32, tag="rr", name="rr")
                    rr_dst = bass.AP(
                        tensor=rrow.tensor, offset=rrow.offset,
                        ap=[[4, 128], [512, 2], [1, 4]])
                    nc.sync.dma_start(rr_dst, rout)
                    bcs = bcp.tile([64, 2, 512], F32, tag="bc", name="bc")
                    bc_src = bass.AP(
                        tensor=rrow.tensor, offset=rrow.offset,
                        ap=[[0, 64], [512, 2], [1, 512]])
                    nc.sync.dma_start(bcs, bc_src)
                    state["bcs"] = bcs

                def muls():
                    ov, bcs = state["ov"], state["bcs"]
                    for hh in range(2):
                        oh = hh * 64
                        nc.vector.tensor_mul(
                            otn[oh:oh + 64, p, qsl],
                            ov[0:64, hh * 512:(hh + 1) * 512],
                            bcs[:, hh, :])

                deferred.append((s_end + 2, evac))
                deferred.append((s_end + 3, recip))
                deferred.append((s_end + 5, muls))

            with tc.tile_pool(name="sm", bufs=4) as sm, \
                 tc.tile_pool(name="stp", bufs=3, space="PSUM") as stp, \
                 tc.tile_pool(name="pvp", bufs=1, space="PSUM") as pvp:
                # pair-batched emission: 4 QK matmuls (64-row-tiled mode,
                # h0/h64 tiles run concurrently) then 4 PV matmuls (full
                # 128x128 mode) — mode switches per pair instead of per
                # matmul, avoiding a PE drain on every instruction.  Each
                # step's two head-halves share one 2-bank st/pt tile so a
                # single exp covers both and the st ring gives 3 steps of
                # slack (QK(s) waits only exp(s-3), keeping QKs adjacent).
                assert len(steps) % 2 == 0
                prev_pair = []  # list of (pt, p, kb, first, last, pv, rel, w)
                for k2 in range(0, n_steps, 2):
                    pair = steps[k2:k2 + 2]
                    emitted = []
                    for (j, p, kb, first, last) in pair:
                        if first:
                            pv = pvp.tile([65, 1024], F32, tag="pv",
                                          name="pv")
                        rel = max(kb * 128 - j * 512, 0)
                        w = 512 - rel
                        qsl = slice(j * 512 + rel, (j + 1) * 512)
                        st = stp.tile([128, 1024], F32, tag="st", name="st")
                        for hh in range(2):
                            oh = hh * 64
                            nc.tensor.matmul(
                                st[:, hh * 512:hh * 512 + w],
                                (kt[oh:oh + 64, p, kb * 128:(kb + 1) * 128]),
                                (qt[oh:oh + 64, p, qsl]),
                                start=True, stop=True)
                        emitted.append((j, p, kb, first, last, pv, rel, w,
                                        st))
                    # previous pair's PVs (full-array mode, one switch)
                    for pr in prev_pair:
                        _emit_pv(nc, va, pr)
                    run_due(k2 + 1)
                    prev_pair = []
                    for (j, p, kb, first, last, pv, rel, w, st) in emitted:
                        diag = kb * 128 - j * 512 >= 0
                        if diag:
                            # causal staircase bias, first 128 cols of each
                            # head-half
                            for hh in range(2):
                                sl = st[:, hh * 512:hh * 512 + 128]
                                nc.vector.tensor_add(sl, sl,
                                                     tri_t[:, 512:640])
                        pt = sm.tile([128, 1024], BF16, tag="pt", name="pt")
                        # one exp covers both head-halves ([w:512] of the
                        # first half is stale psum; bounded, never read)
                        nc.scalar.activation(pt[:, 0:512 + w],
                                             st[:, 0:512 + w], AF.Exp)
                        prev_pair.append((pt, p, kb, first, last, pv, rel,
                                          w))
                        if last:
                            make_normalize(j, p, pv, k2 + 1)
                for pr in prev_pair:
                    _emit_pv(nc, va, pr)
                # flush everything due so far (incl. the last unit's psum
                # evac) while the score/pv pools are still open
                run_due(n_steps + 1)

            # ---- phase 3: output projection (partial; host reduces) ----
            with tc.tile_pool(name="ost2", bufs=4) as ost2, \
                 tc.tile_pool(name="ops", bufs=4, space="PSUM") as ops:
                for t in range(NT):
                    if t == 4:
                        run_due(n_steps + 3)
                    if t == 8:
                        run_due(n_steps + 5)
                    # full 4KB output rows per DMA packet: evac both
                    # 512-wide psum halves into one [128, 1024] tile
                    so = ost2.tile([128, 1024], F32, tag="so", name="so")
                    for n in range(2):
                        ps = ops.tile([128, 512], F32, tag="ops", name="ops")
                        for c2 in range(2):
                            nc.tensor.matmul(
                                ps, (otn[:, c2, t * 128:(t + 1) * 128]),
                                (wo_t[:, c2, n * 512:(n + 1) * 512]),
                                start=(c2 == 0), stop=(c2 == 1))
                        nc.vector.tensor_copy(so[:, n * 512:(n + 1) * 512],
                                              ps)
                    nc.sync.dma_start(out[t * 128:(t + 1) * 128, :], so)

    nc.compile()
    return nc


def _emit_pv(nc, va, prev):
    pt, p, kb, first, last, pv, rel, w = prev
    for hh in range(2):
        h = 2 * p + hh
        nc.tensor.matmul(
            pv[:, hh * 512 + rel:(hh + 1) * 512],
            (va[:, kb, h * 65:h * 65 + 65]),
            (pt[:, hh * 512:hh * 512 + w]),
            start=first, stop=last, skip_group_check=True)


def _tri_np():
    # staircase causal bias: tri[kk, x] = NEG if x < 512+kk else 0
    xs = np.arange(1024)[None, :]
    ks = np.arange(128)[:, None]
    return np.where(xs < 512 + ks, np.float32(NEG),
                    np.float32(0.0)).astype(np.float32)


def build_in_maps(x, Wq, bq, Wk, bk, Wv, bv, Wo):
    import ml_dtypes
    bf16 = ml_dtypes.bfloat16
    tri_np = _tri_np()
    xT_b = [np.ascontiguousarray(x[b].T).astype(bf16) for b in range(B)]
    in_maps = []
    for c in range(N_CORES):
        b, tp = divmod(c, TPG)
        sl = slice(tp * CH, (tp + 1) * CH)
        # V bias is folded into the host-side output bias (bv @ Wo); the
        # ones columns are memset on-device.
        wv_aug = np.zeros((D, CHA), dtype=np.float32)
        for h in range(HPC):
            hsl = slice(tp * CH + h * DH, tp * CH + (h + 1) * DH)
            wv_aug[:, h * 65:h * 65 + DH] = Wv[:, hsl]

        def chunked(w, nch):
            # [nch*128, cols] -> [128, nch*cols]: per-partition rows hold
            # all chunks contiguously so DMA packets are full 4KB rows
            cols = w.shape[1]
            return np.ascontiguousarray(
                w.reshape(nch, 128, cols).transpose(1, 0, 2)
            ).reshape(128, nch * cols).astype(bf16)

        in_maps.append({
            "xT": xT_b[b],
            "wq": chunked(np.ascontiguousarray(Wq[:, sl]), 8),
            "wk": chunked(np.ascontiguousarray(Wk[:, sl]), 8),
            "wv": chunked(wv_aug, 8),
            "wo": chunked(np.ascontiguousarray(Wo[sl, :]), 2),
            "bq": (bq[sl].astype(np.float32) * 0.125).reshape(2, 128).T.copy(),
            "bk": bk[sl].astype(np.float32).reshape(2, 128).T.copy(),
            "tri": tri_np,
            "onesf": np.ones((1, 64), dtype=np.float32),
        })
    return in_maps


def _get_program():
    global _PROG
    if _PROG is None:
        _PROG = _build_program()
    return _PROG


def kernel(x, mask, Wq, bq, Wk, bk, Wv, bv, Wo, bo):
    x = np.asarray(x, dtype=np.float32)
    mask = np.asarray(mask)
    Wq, Wk, Wv, Wo = (np.asarray(w, dtype=np.float32)
                      for w in (Wq, Wk, Wv, Wo))
    bq, bk, bv, bo = (np.asarray(b, dtype=np.float32)
                      for b in (bq, bk, bv, bo))
    causal = bool(
        np.array_equal(mask != 0,
                       np.tril(np.ones((S, S), dtype=bool))))
    if not causal:
        # Fallback for non-causal masks: exact host computation.
        q = (x @ Wq + bq).reshape(B, S, H, DH).transpose(0, 2, 1, 3)
        k = (x @ Wk + bk).reshape(B, S, H, DH).transpose(0, 2, 1, 3)
        v = (x @ Wv + bv).reshape(B, S, H, DH).transpose(0, 2, 1, 3)
        attn = np.einsum("bhqd,bhkd->bhqk", q, k) / np.sqrt(np.float32(DH))
        attn = np.where(mask == 0, np.float32(-1e9), attn)
        attn = attn - attn.max(axis=-1, keepdims=True)
        e = np.exp(attn)
        p = e / e.sum(axis=-1, keepdims=True)
        o = np.einsum("bhqk,bhkd->bhqd", p, v)
        o = o.transpose(0, 2, 1, 3).reshape(B, S, D)
        return (o @ Wo + bo).astype(np.float32)

    nc = _get_program()
    in_maps = build_in_maps(x, Wq, bq, Wk, bk, Wv, bv, Wo)
    res = run_bass_kernel_spmd(nc, in_maps, core_ids=list(range(N_CORES)))
    out = np.zeros((B, S, D), dtype=np.float32)
    for c in range(N_CORES):
        out[c // TPG] += res.results[c]["out"]
    # softmax weights sum to 1, so the V bias contributes exactly bv @ Wo
    out += (bv.astype(np.float32) @ Wo) + bo.astype(np.float32)
    return out


# revision 33
# speedup vs baseline: 1.3613x; 1.0244x over previous
"""Multi-head causal self-attention (B=2, S=2048, D=1024, H=16) on 8 TRN2 NeuronCores.

Sharding: data-parallel over batch (2) x tensor-parallel over heads (4 groups of
4 heads). Each core computes Q/K/V projections for its 4 heads, causal
flash-style attention (scores kept transposed [k, q] so no on-chip transposes
are needed), and a partial output projection against its row-slice of W_O.
Host sums the 4 partials per batch and adds the output bias.

v2: bf16 matmul operands (less PE power -> less duty-cycle throttle, half the
DMA/LDWEIGHTS bytes), per-k-block software-pipelined attention steps (the PE
queue is [QK(s), PV(s-1)] so PV never waits on the softmax exp), PSUM
rebalanced (4x one-bank score tiles + 2x double-head pv accumulators), and the
softmax normalize chain (psum evac -> den bounce -> [128,8] reciprocal ->
stride-0 broadcast -> multiply) deferred several steps off the critical path.
"""

import contextlib
import sys

import numpy as np

sys.path.insert(0, "/opt/trn_rl_repo")

import concourse.bass as bass  # noqa: E402
import concourse.tile as tile  # noqa: E402
from concourse import bacc, mybir  # noqa: E402
from concourse.bass_utils import run_bass_kernel_spmd  # noqa: E402

F32 = mybir.dt.float32
BF16 = mybir.dt.bfloat16
AF = mybir.ActivationFunctionType
ALU = mybir.AluOpType

B, S, D, H = 2, 2048, 1024, 16
DH = D // H          # 64
TPG = 4              # tensor-parallel groups
HPC = H // TPG       # 4 heads per core
CH = HPC * DH        # 256 channels per core
CHA = CH + HPC       # 260: V channels augmented with a ones column per head
NEG = -1.0e9
N_CORES = 8

_PROG = None  # cached compiled Bass program


def _build_program():
    nc = bacc.Bacc("TRN2", target_bir_lowering=False, debug=False,
                   num_devices=N_CORES)

    # weights arrive chunk-interleaved ([partition, chunk, col]) so each
    # DMA packet is a full 4KB per-partition row
    xT = nc.dram_tensor("xT", [D, S], BF16, kind="ExternalInput").ap()
    wq = nc.dram_tensor("wq", [128, 8 * CH], BF16, kind="ExternalInput").ap()
    wk = nc.dram_tensor("wk", [128, 8 * CH], BF16, kind="ExternalInput").ap()
    wv = nc.dram_tensor("wv", [128, 8 * CHA], BF16, kind="ExternalInput").ap()
    wo = nc.dram_tensor("wo", [128, 2 * D], BF16, kind="ExternalInput").ap()
    bq = nc.dram_tensor("bq", [128, 2], F32, kind="ExternalInput").ap()
    bk = nc.dram_tensor("bk", [128, 2], F32, kind="ExternalInput").ap()
    tri = nc.dram_tensor("tri", [128, 1024], F32, kind="ExternalInput").ap()
    onesf = nc.dram_tensor("onesf", [1, 64], F32, kind="ExternalInput").ap()
    out = nc.dram_tensor("out", [S, D], F32, kind="ExternalOutput").ap()

    NQ = S // 512    # 4 q-blocks of 512
    NT = S // 128    # 16 s-tiles / k-blocks

    with tile.TileContext(nc) as tc, contextlib.ExitStack() as ctx:
        const = ctx.enter_context(tc.tile_pool(name="const", bufs=1))
        # Q^T/8 (+bq/8) and K^T (+bk), one tile per (head-pair chunk m,
        # 512-col block n) so dependency tracking is exact
        qt = [[const.tile([128, 512], BF16, name=f"qt{m}_{n}")
               for n in range(S // 512)] for m in range(2)]
        kt = [[const.tile([128, 512], BF16, name=f"kt{m}_{n}")
               for n in range(S // 512)] for m in range(2)]
        va = const.tile([128, NT, CHA], BF16)  # V augmented: [s, head-major 65-col blocks]
        otn = const.tile([128, 2, S], BF16)    # normalized attention out, transposed
        tri_t = const.tile([128, 1024], F32)
        ones64f = const.tile([1, 64], F32)
        bq_t = const.tile([128, 2], F32)
        bk_t = const.tile([128, 2], F32)
        wo_t = const.tile([128, 2, D], BF16)

        # ---- phase 1: projections --------------------------------------
        with tc.tile_pool(name="proj", bufs=1) as proj, \
             tc.tile_pool(name="pqk", bufs=8, space="PSUM") as pqk:
            xt = proj.tile([128, 8, S], BF16)
            wq_t = proj.tile([128, 8, CH], BF16)
            wk_t = proj.tile([128, 8, CH], BF16)
            wv_t = proj.tile([128, 8, CHA], BF16)
            xTr = xT.rearrange("(a p) s -> a p s", p=128)
            # x streams chunk-by-chunk (4KB per-partition packets); the
            # critical leading transfers are split across several DMA
            # rings so the first matmul group is gated on ~256KB only
            for q4 in range(4):
                nc.sync.dma_start(wq_t[:, 2 * q4:2 * q4 + 2, :],
                                  wq[:, q4 * 512:(q4 + 1) * 512])
            for n in range(NQ):
                nc.sync.dma_start(xt[:, 0, n * 512:(n + 1) * 512],
                                  xTr[0][:, n * 512:(n + 1) * 512])
            nc.sync.dma_start(bq_t, bq)
            for q4 in range(4):
                nc.sync.dma_start(wk_t[:, 2 * q4:2 * q4 + 2, :],
                                  wk[:, q4 * 512:(q4 + 1) * 512])
            nc.sync.dma_start(bk_t, bk)
            for c in range(1, 8):
                nc.sync.dma_start(xt[:, c, :], xTr[c])
            nc.sync.dma_start(wv_t, wv)
            # non-projection constants after the projection-critical stream
            nc.sync.dma_start(ones64f, onesf)
            nc.sync.dma_start(tri_t, tri)
            nc.sync.dma_start(wo_t, wo)

            # preload the ACT exp table set while ACT is otherwise idle
            nc.scalar.activation(ones64f, ones64f, AF.Exp)

            def qk_pass(m):
                # c-outer: 8 open psum groups (Q n0..3, K n0..3) accumulate
                # as each x chunk arrives; first matmul needs only chunk 0
                groups = []
                for w_t, dst, bias_t, scaled in ((wq_t, qt, bq_t, True),
                                                 (wk_t, kt, bk_t, False)):
                    for n in range(NQ):
                        ps = pqk.tile([128, 512], F32, tag="pqk", name="pqk")
                        groups.append((ps, w_t, dst, bias_t, scaled, n))
                for c in range(8):
                    for gi, (ps, w_t, dst, bias_t, scaled, n) in \
                            enumerate(groups):
                        nc.tensor.matmul(
                            ps, (w_t[:, c, m * 128:(m + 1) * 128]),
                            (xt[:, c, n * 512:(n + 1) * 512]),
                            start=(c == 0), stop=(c == 7))
                        if c == 7:
                            # evacuate, folding 1/sqrt(DH) into Q; split
                            # between ACT and DVE so the burst of 8 drains
                            # 2x faster
                            bias_ap = bias_t[:, m:m + 1]
                            if gi % 2 == 0:
                                nc.scalar.activation(
                                    dst[m][n], ps, AF.Identity,
                                    bias=bias_ap,
                                    scale=0.125 if scaled else 1.0)
                            elif scaled:
                                nc.vector.tensor_scalar(
                                    dst[m][n], ps, 0.125, bias_ap,
                                    ALU.mult, ALU.add)
                            else:
                                nc.vector.tensor_scalar_add(
                                    dst[m][n], ps, bias_ap)

            def v_proj():
                # V (not transposed): stationary = x^T tile, moving = wv_aug
                # (the V bias folds into the host-side output bias because
                # softmax weights sum to 1: o_norm += bv  =>  out += bv @ Wo)
                for t in range(NT):
                    psv = pqk.tile([128, CHA], F32, tag="pqk", name="psv")
                    for c in range(8):
                        nc.tensor.matmul(
                            psv, (xt[:, c, t * 128:(t + 1) * 128]),
                            (wv_t[:, c, :]), start=(c == 0), stop=(c == 7))
                    nc.vector.tensor_copy(va[:, t, :], psv)
                    # the per-head denominator "ones" columns (65th of each
                    # head block; zero in wv_aug so psv has zeros there)
                    ones_view = bass.AP(
                        tensor=va.tensor, offset=va.offset + t * CHA + DH,
                        ap=[list(va.ap)[0], [DH + 1, HPC]])
                    nc.vector.memset(ones_view, 1.0)

            qk_pass(0)
            v_proj()
            qk_pass(1)

        # ---- phase 2: attention, software-pipelined per k-block --------
        # step = (j, p, kb): one 128-row k-block of scores for a 512-col
        # q-block, both packed head-halves (hh).  The PE queue is
        # [QK(s), PV(s-1)] so PV never waits on exp latency.  The softmax
        # normalize chain for each (j, p) unit is deferred several steps.
        steps = []
        for j in range(NQ):
            for p in range(2):
                nkb = 4 * (j + 1)
                for kb in range(nkb):
                    steps.append((j, p, kb, kb == 0, kb == nkb - 1))
        n_steps = len(steps)

        deferred = []   # (due_step, fn), kept sorted by due step

        def run_due(s):
            while deferred and deferred[0][0] <= s:
                deferred.pop(0)[1]()

        def defer(due, fn):
            deferred.append((due, fn))
            deferred.sort(key=lambda d: d[0])

        with tc.tile_pool(name="ovp", bufs=2) as ovp, \
             tc.tile_pool(name="rcp", bufs=2) as rcp, \
             tc.tile_pool(name="bcp", bufs=2) as bcp, \
             tc.tile_pool(name="dsp", bufs=4, space="DRAM") as dsp:

            def make_normalize(j, p, pv, s_end):
                # chain: evac pv->SBUF (frees PSUM) -> bounce den ->
                # [128,8] reciprocal -> bounce back -> stride-0 broadcast
                # -> per-head multiply into otn.
                qsl = slice(j * 512, (j + 1) * 512)
                state = {}

                def evac():
                    ov = ovp.tile([65, 1024], F32, tag="ov", name="ov")
                    nc.vector.tensor_copy(ov, pv)
                    drow = dsp.tile([1, 1024], F32, tag="ds", name="ds")
                    nc.sync.dma_start(drow, ov[64:65, :])
                    rin = rcp.tile([128, 2, 4], F32, tag="ri", name="ri")
                    din_src = bass.AP(
                        tensor=drow.tensor, offset=drow.offset,
                        ap=[[4, 128], [512, 2], [1, 4]])
                    nc.sync.dma_start(rin, din_src)
                    state["ov"] = ov
                    state["rin"] = rin

                def recip():
                    rout = rcp.tile([128, 2, 4], F32, tag="ro", name="ro")
                    nc.vector.reciprocal(rout, state["rin"])
                    rrow = dsp.tile([1, 1024], F32, tag="rr", name="rr")
                    rr_dst = bass.AP(
                        tensor=rrow.tensor, offset=rrow.offset,
                        ap=[[4, 128], [512, 2], [1, 4]])
                    nc.sync.dma_start(rr_dst, rout)
                    bcs = bcp.tile([64, 2, 512], F32, tag="bc", name="bc")
                    bc_src = bass.AP(
                        tensor=rrow.tensor, offset=rrow.offset,
                        ap=[[0, 64], [512, 2], [1, 512]])
                    nc.sync.dma_start(bcs, bc_src)
                    state["bcs"] = bcs

                def muls():
                    # on GpSimd (all-SBUF operands): a DMA-gated wait here
                    # must not block the DVE queue's causal tri-adds
                    ov, bcs = state["ov"], state["bcs"]
                    for hh in range(2):
                        oh = hh * 64
                        nc.gpsimd.tensor_mul(
                            otn[oh:oh + 64, p, qsl],
                            ov[0:64, hh * 512:(hh + 1) * 512],
                            bcs[:, hh, :])

                defer(s_end + 2, evac)
                defer(s_end + 3, recip)
                defer(s_end + 7, muls)

            with tc.tile_pool(name="sm", bufs=4) as sm, \
                 tc.tile_pool(name="stp", bufs=3, space="PSUM") as stp, \
                 tc.tile_pool(name="pvp", bufs=1, space="PSUM") as pvp:
                # pair-batched emission: 4 QK matmuls (64-row-tiled mode,
                # h0/h64 tiles run concurrently) then 4 PV matmuls (full
                # 128x128 mode) — mode switches per pair instead of per
                # matmul, avoiding a PE drain on every instruction.  Each
                # step's two head-halves share one 2-bank st/pt tile so a
                # single exp covers both and the st ring gives 3 steps of
                # slack (QK(s) waits only exp(s-3), keeping QKs adjacent).
                assert len(steps) % 2 == 0
                prev_pair = []  # list of (pt, p, kb, first, last, pv, rel, w)
                for k2 in range(0, n_steps, 2):
                    pair = steps[k2:k2 + 2]
                    emitted = []
                    for (j, p, kb, first, last) in pair:
                        if first:
                            pv = pvp.tile([65, 1024], F32, tag="pv",
                                          name="pv")
                        rel = max(kb * 128 - j * 512, 0)
                        w = 512 - rel
                        kq = (kb % 4) * 128
                        st = stp.tile([128, 1024], F32, tag="st", name="st")
                        for hh in range(2):
                            oh = hh * 64
                            nc.tensor.matmul(
                                st[:, hh * 512:hh * 512 + w],
                                (kt[p][kb // 4][oh:oh + 64, kq:kq + 128]),
                                (qt[p][j][oh:oh + 64, rel:512]),
                                start=True, stop=True)
                        emitted.append((j, p, kb, first, last, pv, rel, w,
                                        st))
                    # previous pair's PVs (full-array mode, one switch)
                    for pr in prev_pair:
                        _emit_pv(nc, va, pr)
                    run_due(k2 + 1)
                    prev_pair = []
                    for (j, p, kb, first, last, pv, rel, w, st) in emitted:
                        diag = kb * 128 - j * 512 >= 0
                        if diag:
                            # causal staircase bias, first 128 cols of each
                            # head-half
                            for hh in range(2):
                                sl = st[:, hh * 512:hh * 512 + 128]
                                nc.vector.tensor_add(sl, sl,
                                                     tri_t[:, 512:640])
                        pt = sm.tile([128, 1024], BF16, tag="pt", name="pt")
                        # one exp covers both head-halves ([w:512] of the
                        # first half is stale psum; bounded, never read)
                        nc.scalar.activation(pt[:, 0:512 + w],
                                             st[:, 0:512 + w], AF.Exp)
                        prev_pair.append((pt, p, kb, first, last, pv, rel,
                                          w))
                        if last:
                            make_normalize(j, p, pv, k2 + 1)
                for pr in prev_pair:
                    _emit_pv(nc, va, pr)
                # flush everything due so far (incl. the last unit's psum
                # evac) while the score/pv pools are still open
                run_due(n_steps + 1)

            # ---- phase 3: output projection (partial; host reduces) ----
            with tc.tile_pool(name="ost2", bufs=4) as ost2, \
                 tc.tile_pool(name="ops", bufs=4, space="PSUM") as ops:
                for t in range(NT):
                    if t == 4:
                        run_due(n_steps + 4)
                    if t == 8:
                        run_due(n_steps + 8)
                    # full 4KB output rows per DMA packet: evac both
                    # 512-wide psum halves into one [128, 1024] tile
                    so = ost2.tile([128, 1024], F32, tag="so", name="so")
                    for n in range(2):
                        ps = ops.tile([128, 512], F32, tag="ops", name="ops")
                        for c2 in range(2):
                            nc.tensor.matmul(
                                ps, (otn[:, c2, t * 128:(t + 1) * 128]),
                                (wo_t[:, c2, n * 512:(n + 1) * 512]),
                                start=(c2 == 0), stop=(c2 == 1))
                        nc.vector.tensor_copy(so[:, n * 512:(n + 1) * 512],
                                              ps)
                    nc.sync.dma_start(out[t * 128:(t + 1) * 128, :], so)

    nc.compile()
    return nc


def _emit_pv(nc, va, prev):
    pt, p, kb, first, last, pv, rel, w = prev
    for hh in range(2):
        h = 2 * p + hh
        nc.tensor.matmul(
            pv[:, hh * 512 + rel:(hh + 1) * 512],
            (va[:, kb, h * 65:h * 65 + 65]),
            (pt[:, hh * 512:hh * 512 + w]),
            start=first, stop=last, skip_group_check=True)


def _tri_np():
    # staircase causal bias: tri[kk, x] = NEG if x < 512+kk else 0
    xs = np.arange(1024)[None, :]
    ks = np.arange(128)[:, None]
    return np.where(xs < 512 + ks, np.float32(NEG),
                    np.float32(0.0)).astype(np.float32)


def build_in_maps(x, Wq, bq, Wk, bk, Wv, bv, Wo):
    import ml_dtypes
    bf16 = ml_dtypes.bfloat16
    tri_np = _tri_np()
    xT_b = [np.ascontiguousarray(x[b].T).astype(bf16) for b in range(B)]
    in_maps = []
    for c in range(N_CORES):
        b, tp = divmod(c, TPG)
        sl = slice(tp * CH, (tp + 1) * CH)
        # V bias is folded into the host-side output bias (bv @ Wo); the
        # ones columns are memset on-device.
        wv_aug = np.zeros((D, CHA), dtype=np.float32)
        for h in range(HPC):
            hsl = slice(tp * CH + h * DH, tp * CH + (h + 1) * DH)
            wv_aug[:, h * 65:h * 65 + DH] = Wv[:, hsl]

        def chunked(w, nch):
            # [nch*128, cols] -> [128, nch*cols]: per-partition rows hold
            # all chunks contiguously so DMA packets are full 4KB rows
            cols = w.shape[1]
            return np.ascontiguousarray(
                w.reshape(nch, 128, cols).transpose(1, 0, 2)
            ).reshape(128, nch * cols).astype(bf16)

        in_maps.append({
            "xT": xT_b[b],
            "wq": chunked(np.ascontiguousarray(Wq[:, sl]), 8),
            "wk": chunked(np.ascontiguousarray(Wk[:, sl]), 8),
            "wv": chunked(wv_aug, 8),
            "wo": chunked(np.ascontiguousarray(Wo[sl, :]), 2),
            "bq": (bq[sl].astype(np.float32) * 0.125).reshape(2, 128).T.copy(),
            "bk": bk[sl].astype(np.float32).reshape(2, 128).T.copy(),
            "tri": tri_np,
            "onesf": np.ones((1, 64), dtype=np.float32),
        })
    return in_maps


def _get_program():
    global _PROG
    if _PROG is None:
        _PROG = _build_program()
    return _PROG


def kernel(x, mask, Wq, bq, Wk, bk, Wv, bv, Wo, bo):
    x = np.asarray(x, dtype=np.float32)
    mask = np.asarray(mask)
    Wq, Wk, Wv, Wo = (np.asarray(w, dtype=np.float32)
                      for w in (Wq, Wk, Wv, Wo))
    bq, bk, bv, bo = (np.asarray(b, dtype=np.float32)
                      for b in (bq, bk, bv, bo))
    causal = bool(
        np.array_equal(mask != 0,
                       np.tril(np.ones((S, S), dtype=bool))))
    if not causal:
        # Fallback for non-causal masks: exact host computation.
        q = (x @ Wq + bq).reshape(B, S, H, DH).transpose(0, 2, 1, 3)
        k = (x @ Wk + bk).reshape(B, S, H, DH).transpose(0, 2, 1, 3)
        v = (x @ Wv + bv).reshape(B, S, H, DH).transpose(0, 2, 1, 3)
        attn = np.einsum("bhqd,bhkd->bhqk", q, k) / np.sqrt(np.float32(DH))
        attn = np.where(mask == 0, np.float32(-1e9), attn)
        attn = attn - attn.max(axis=-1, keepdims=True)
        e = np.exp(attn)
        p = e / e.sum(axis=-1, keepdims=True)
        o = np.einsum("bhqk,bhkd->bhqd", p, v)
        o = o.transpose(0, 2, 1, 3).reshape(B, S, D)
        return (o @ Wo + bo).astype(np.float32)

    nc = _get_program()
    in_maps = build_in_maps(x, Wq, bq, Wk, bk, Wv, bv, Wo)
    res = run_bass_kernel_spmd(nc, in_maps, core_ids=list(range(N_CORES)))
    out = np.zeros((B, S, D), dtype=np.float32)
    for c in range(N_CORES):
        out[c // TPG] += res.results[c]["out"]
    # softmax weights sum to 1, so the V bias contributes exactly bv @ Wo
    out += (bv.astype(np.float32) @ Wo) + bo.astype(np.float32)
    return out
